# revision 13
# baseline (speedup 1.0000x reference)
"""Trainium2 Bass kernel for a 2-layer LSTM decoder (B=512, T=128, H=1024).

Strategy:
  - Batch is sharded 4 ways (128 rows per core group); cores 4-7 run the
    same program on duplicated batch groups (the harness reads cores 0-3).
    This keeps every DoubleRow matmul writing a full [128,512] PSUM tile
    at partition offset 0 (the fp8 DoubleRow ISA mode does not support
    PSUM column-tile offsets), while per-instruction cost is unchanged --
    DR cost is out_free * 0.5 cycles regardless of batch width.
  - All recurrent GEMMs run in fp8-e4m3 DoubleRow: lhsT [128,2,128] h^T
    chunk pairs stationary, rhs [128,2,512] weight pairs moving; each
    instruction contracts 256 hidden dims at 0.5 PE cycles/element.
  - fp8 operands (weights, h, x) are pre-scaled by 16 to dodge e4m3
    subnormals; the 1/256 gate scale folds into the activation `scale`.
  - Gates are written to SBUF as bf16 (DVE 2x SBUF perf mode); cell state
    c stays fp32. Batch is the partition dim everywhere (no stacking).
  - h returns to h^T each step with DMA xbar transposes (bf16) + a DVE
    cast-with-scale to fp8.
  - The output projection is 4 DR matmuls with W_out^T stationary over
    fp8 h1^T pairs, yielding out^T [1,128] directly; the MSE-loss tail
    is computed on the host in fp32.
"""

import os

import numpy as np
import ml_dtypes

import concourse.bass as bass
import concourse.tile as tile
import concourse.mybir as mybir

BF16 = ml_dtypes.bfloat16
FP8 = ml_dtypes.float8_e4m3
N_CORES = 8
N_GROUPS = 4
B, T_FULL, H = 512, 128, 1024
BL = B // N_GROUPS  # 128 batch rows per core group
AF = mybir.ActivationFunctionType
DT = mybir.dt
DR = mybir.MatmulPerfMode.DoubleRow
SC = 16.0  # fp8 operand pre-scale

_T = int(os.environ.get("LSTM_KERNEL_T", str(T_FULL)))


def _split_multi_waits(nc):
    """walrus in this container supports only ONE sync wait per instruction.
    Move extra waits onto preceding same-engine NOPs (engine FIFO makes this
    semantically identical)."""
    for f in nc.m.functions:
        for bb in f.blocks:
            new = []
            for ins in bb.instructions:
                si = ins.sync_info
                if si is not None and si.on_wait and len(si.on_wait) > 1:
                    waits = list(si.on_wait)
                    for w in waits[:-1]:
                        nop = mybir.InstNoOp(
                            name=nc.get_next_instruction_name(), ins=[], outs=[]
                        )
                        nop.engine = ins.engine
                        nop.sync_info = mybir.SyncInfo(on_wait=[w], on_update=[])
                        nc.register_instruction(nop)
                        new.append(nop)
                    si.on_wait = [waits[-1]]
                new.append(ins)
            bb.instructions = new


def _build_program(t_steps):
    nc = bass.Bass(dynamic_dma_scratch_size=512)
    F8 = DT.float8e4

    w0p_d = nc.dram_tensor("W0P", [128, 4, 2, 4096], F8, kind="ExternalInput")
    w1p_d = nc.dram_tensor("W1P", [128, 8, 2, 4096], F8, kind="ExternalInput")
    w0xb_d = nc.dram_tensor("W0XB", [1, 2, 4096], F8, kind="ExternalInput")
    w1b_d = nc.dram_tensor("W1B", [1, 2, 4096], F8, kind="ExternalInput")
    wout_d = nc.dram_tensor("WOUT8", [128, 8, 4], F8, kind="ExternalInput")
    bouts_d = nc.dram_tensor("BOUTS", [1, 2], DT.float32, kind="ExternalInput")
    ht_d = nc.dram_tensor("HT8", [128, 16, 128], F8, kind="ExternalInput")
    cs_d = nc.dram_tensor("CS", [128, 2, 1024], DT.float32, kind="ExternalInput")
    xin_d = nc.dram_tensor("XIN8", [1, 2, 128], F8, kind="ExternalInput")
    ones_d = nc.dram_tensor("ONES16", [1, 2, 128], F8, kind="ExternalInput")
    outd = nc.dram_tensor("OUTD", [T_FULL, BL], DT.float32, kind="ExternalOutput")

    with tile.TileContext(nc) as tc:
        with (
            tc.tile_pool(name="const", bufs=1) as const,
            tc.tile_pool(name="psum", bufs=4, space="PSUM") as psum,
            tc.tile_pool(name="gt", bufs=12) as gt,
            tc.tile_pool(name="drow", bufs=2) as drowp,
        ):
            w0p = const.tile([128, 4, 2, 4096], F8)
            w1p = const.tile([128, 8, 2, 4096], F8)
            w0xb = const.tile([1, 2, 4096], F8)
            w1b = const.tile([1, 2, 4096], F8)
            wout = const.tile([128, 8, 4], F8)
            bouts = const.tile([1, 2], DT.float32)
            ht = const.tile([128, 16, 128], F8)
            htb = const.tile([128, 8, 128], DT.bfloat16)
            cs = const.tile([128, 2, 1024], DT.float32)
            xch8 = const.tile([1, 2, 128], F8)
            ones16 = const.tile([1, 2, 128], F8)
            hs0 = const.tile([128, 1024], DT.bfloat16)
            hs1 = const.tile([128, 1024], DT.bfloat16)

            nc.sync.dma_start(w0p[:], w0p_d[:])
            nc.sync.dma_start(w1p[:], w1p_d[:])
            nc.sync.dma_start(w0xb[:], w0xb_d[:])
            nc.sync.dma_start(w1b[:], w1b_d[:])
            nc.sync.dma_start(wout[:], wout_d[:])
            nc.sync.dma_start(bouts[:], bouts_d[:])
            nc.sync.dma_start(ht[:], ht_d[:])
            nc.sync.dma_start(cs[:], cs_d[:])
            nc.sync.dma_start(xch8[:], xin_d[:])
            nc.sync.dma_start(ones16[:], ones_d[:])

            def htp(layer, kp):
                """h^T DoubleRow pair AP [128, 2, 128]; plane i = hidden
                chunk 2*kp+i of the layer."""
                return ht[:, 8 * layer + 2 * kp : 8 * layer + 2 * kp + 2, :]

            def elt(pr, l, hs):
                """LSTM elementwise for layer l: 4 PSUM gate-pair tiles
                [128,2,512] (i,f,g,o; x256 scale) -> h (bf16), c in place.
                Each activation reads a full pair (2 banks) in one instr."""
                csl = cs[:, l, :]
                gi = gt.tile([128, 1024], DT.bfloat16, tag="gt")
                gf = gt.tile([128, 1024], DT.bfloat16, tag="gt")
                gg = gt.tile([128, 1024], DT.bfloat16, tag="gt")
                go = gt.tile([128, 1024], DT.bfloat16, tag="gt")
                nc.scalar.activation(gi[:], pr[0][:, :, :], AF.Sigmoid, scale=1.0 / 256)
                nc.scalar.activation(gg[:], pr[2][:, :, :], AF.Tanh, scale=1.0 / 256)
                nc.scalar.activation(gf[:], pr[1][:, :, :], AF.Sigmoid, scale=1.0 / 256)
                nc.scalar.activation(go[:], pr[3][:, :, :], AF.Sigmoid, scale=1.0 / 256)
                t1 = gt.tile([128, 1024], DT.bfloat16, tag="gt")
                nc.vector.tensor_mul(t1[:], gi[:], gg[:])
                nc.vector.tensor_mul(csl, gf[:], csl)
                nc.vector.tensor_add(csl, csl, t1[:])
                thc = gt.tile([128, 1024], DT.bfloat16, tag="gt")
                nc.scalar.activation(thc[:], csl, AF.Tanh)
                nc.vector.tensor_mul(hs[:], go[:], thc[:])

            def gate_tiles(tag):
                """4 gate-pair PSUM tiles [128,2,512] (2 banks each)."""
                return [
                    psum.tile([128, 2, 512], DT.float32, tag="gp", name=f"{tag}_{g}")
                    for g in range(4)
                ]

            for t in range(t_steps):
                # ---- L0 gates: 256*(h0@Whh0 + x@Wih0 + b0)
                pr0 = gate_tiles(f"p0_{t}")
                for kp in range(4):
                    lhsT = htp(0, kp)
                    for ch in range(8):
                        g, n2 = divmod(ch, 2)
                        nc.tensor.matmul(
                            pr0[g][:, n2, :],
                            lhsT,
                            w0p[:, kp, :, 512 * ch : 512 * ch + 512],
                            start=(kp == 0),
                            stop=False,
                            perf_mode=DR,
                        )
                for ch in range(8):
                    g, n2 = divmod(ch, 2)
                    nc.tensor.matmul(
                        pr0[g][:, n2, :],
                        xch8[0:1, :, :],
                        w0xb[0:1, :, 512 * ch : 512 * ch + 512],
                        start=False,
                        stop=True,
                        perf_mode=DR,
                    )

                # ---- L1 part A: h1(t-1)@Whh1 + b1
                pr1 = gate_tiles(f"p1_{t}")
                for kp in range(4):
                    lhsT = htp(1, kp)
                    for ch in range(8):
                        g, n2 = divmod(ch, 2)
                        nc.tensor.matmul(
                            pr1[g][:, n2, :],
                            lhsT,
                            w1p[:, kp, :, 512 * ch : 512 * ch + 512],
                            start=(kp == 0),
                            stop=False,
                            perf_mode=DR,
                        )
                for ch in range(8):
                    g, n2 = divmod(ch, 2)
                    nc.tensor.matmul(
                        pr1[g][:, n2, :],
                        ones16[0:1, :, :],
                        w1b[0:1, :, 512 * ch : 512 * ch + 512],
                        start=False,
                        stop=False,
                        perf_mode=DR,
                    )

                # ---- L0 elementwise -> hs0; transpose + fp8 cast
                elt(pr0, 0, hs0)
                for p in range(8):
                    nc.sync.dma_start_transpose(
                        htb[:, p, :], hs0[:, 128 * p : 128 * (p + 1)]
                    )
                nc.vector.tensor_scalar_mul(ht[:, 0:8, :], htb[:, :, :], SC)

                # ---- L1 part B: h0(t)@Wih1
                for kp in range(4):
                    lhsT = htp(0, kp)
                    for ch in range(8):
                        g, n2 = divmod(ch, 2)
                        nc.tensor.matmul(
                            pr1[g][:, n2, :],
                            lhsT,
                            w1p[:, 4 + kp, :, 512 * ch : 512 * ch + 512],
                            start=False,
                            stop=(kp == 3),
                            perf_mode=DR,
                        )

                # ---- L1 elementwise -> hs1; transpose + fp8 cast
                elt(pr1, 1, hs1)
                for p in range(8):
                    nc.sync.dma_start_transpose(
                        htb[:, p, :], hs1[:, 128 * p : 128 * (p + 1)]
                    )
                nc.vector.tensor_scalar_mul(ht[:, 8:16, :], htb[:, :, :], SC)

                # ---- out^T = 256*(W_out . h1), W_out^T stationary (plain fp8)
                dps = psum.tile([1, BL], DT.float32, tag="gp", name=f"dps_{t}")
                for c in range(8):
                    nc.tensor.matmul(
                        dps[0:1, :],
                        wout[:, c, 0:1],
                        ht[:, 8 + c, :],
                        start=(c == 0),
                        stop=(c == 7),
                    )

                # ---- tail: fp32 row to DRAM, fp8 x*16 for next step
                drow = drowp.tile([1, BL], DT.float32, tag="drow")
                nc.scalar.activation(
                    drow[:], dps[0:1, :], AF.Identity,
                    bias=bouts[0:1, 1:2], scale=1.0 / 256,
                )
                nc.scalar.activation(
                    xch8[0:1, 0:1, :], dps[0:1, :], AF.Identity,
                    bias=bouts[0:1, 0:1], scale=1.0 / 16,
                )
                tw = t % T_FULL
                nc.sync.dma_start(outd[tw : tw + 1, :], drow[:])

    _split_multi_waits(nc)
    return nc


# ---------------------------------------------------------------------------
# host side


def _pairs(wT):
    """[1024, 4096] (contract, gate) -> [128(j), kp, plane, 4096]."""
    return np.ascontiguousarray(
        wT.reshape(4, 2, 128, 4096).transpose(2, 0, 1, 3)
    )


def _prep_shared(inp):
    f32 = np.float32
    w0p = _pairs(np.asarray(inp["W_hh0"], f32).T * SC).astype(FP8)

    w1p = np.zeros((128, 8, 2, 4096), dtype=np.float32)
    w1p[:, 0:4] = _pairs(np.asarray(inp["W_hh1"], f32).T * SC)
    w1p[:, 4:8] = _pairs(np.asarray(inp["W_ih1"], f32).T * SC)

    w0xb = np.zeros((1, 2, 4096), dtype=np.float32)
    w0xb[0, 0] = np.asarray(inp["W_ih0"], f32)[:, 0] * SC
    w0xb[0, 1] = (np.asarray(inp["b_ih0"], f32) + np.asarray(inp["b_hh0"], f32)) * SC

    w1b = np.zeros((1, 2, 4096), dtype=np.float32)
    w1b[0, 0] = (np.asarray(inp["b_ih1"], f32) + np.asarray(inp["b_hh1"], f32)) * SC

    wout = np.zeros((128, 8, 4), dtype=np.float32)
    wout[:, :, 0] = (np.asarray(inp["W_out"], f32)[0] * SC).reshape(8, 128).T

    bouts = np.zeros((1, 2), dtype=np.float32)
    bouts[0, 0] = float(inp["b_out"][0]) * SC
    bouts[0, 1] = float(inp["b_out"][0])

    xin = np.zeros((1, 2, 128), dtype=np.float32)
    xin[0, 1] = SC  # ones plane (x0 = 0)
    ones16 = np.zeros((1, 2, 128), dtype=np.float32)
    ones16[0, 0] = SC

    return {
        "W0P": w0p,
        "W1P": w1p.astype(FP8),
        "W0XB": w0xb.astype(FP8),
        "W1B": w1b.astype(FP8),
        "WOUT8": wout.astype(FP8),
        "BOUTS": bouts,
        "XIN8": xin.astype(FP8),
        "ONES16": ones16.astype(FP8),
    }


def _ht_chunks(h):  # [128(b), 1024] -> [128(j), 8(c), 128(b)]
    return np.ascontiguousarray(
        h.reshape(BL, 8, 128).transpose(2, 1, 0)
    )


def _prep_core(inp, c):
    g = c % N_GROUPS
    sl = slice(BL * g, BL * (g + 1))
    ht = np.zeros((128, 16, 128), dtype=np.float32)
    ht[:, 0:8, :] = _ht_chunks(np.asarray(inp["h0"][0, sl], np.float32)) * SC
    ht[:, 8:16, :] = _ht_chunks(np.asarray(inp["h0"][1, sl], np.float32)) * SC
    cs = np.zeros((128, 2, 1024), dtype=np.float32)
    cs[:, 0, :] = np.asarray(inp["c0"][0, sl], np.float32)
    cs[:, 1, :] = np.asarray(inp["c0"][1, sl], np.float32)
    return {"HT8": ht.astype(FP8), "CS": cs}


_RUNNER = {}


def _get_runner(t_steps):
    """Build the bass program once per process and return a cached callable
    mapping per-core input dicts -> per-core OUTD arrays."""
    if t_steps in _RUNNER:
        return _RUNNER[t_steps]

    import jax
    from jax.sharding import Mesh, PartitionSpec
    from jax.experimental.shard_map import shard_map
    from concourse import bass2jax
    from concourse._compat import axon_active

    nc = _build_program(t_steps)

    if not axon_active():
        from concourse.bass_utils import run_bass_kernel_spmd

        def run_native(in_maps):
            res = run_bass_kernel_spmd(nc, in_maps, list(range(N_CORES)))
            return [r["OUTD"] for r in res.results]

        _RUNNER[t_steps] = run_native
        return run_native

    bass2jax.install_neuronx_cc_hook()

    partition_name = nc.partition_id_tensor.name if nc.partition_id_tensor else None
    in_names = []
    out_names = []
    out_avals = []
    zero_outs = []
    for alloc in nc.m.functions[0].allocations:
        if not isinstance(alloc, mybir.MemoryLocationSet):
            continue
        name = alloc.memorylocations[0].name
        if alloc.kind == "ExternalInput":
            if name != partition_name:
                in_names.append(name)
        elif alloc.kind == "ExternalOutput":
            out_names.append(name)
            shape = tuple(alloc.tensor_shape)
            dtype = mybir.dt.np(alloc.dtype)
            out_avals.append(jax.core.ShapedArray(shape, dtype))
            zero_outs.append(np.zeros(shape, dtype))
    n_params = len(in_names)
    n_outs = len(out_avals)
    all_names = in_names + out_names
    if partition_name is not None:
        all_names = all_names + [partition_name]
    donate = tuple(range(n_params, n_params + n_outs))

    def _body(*args):
        operands = list(args)
        if partition_name is not None:
            operands.append(bass2jax.partition_id_tensor())
        outs = bass2jax._bass_exec_p.bind(
            *operands,
            out_avals=tuple(out_avals),
            in_names=tuple(all_names),
            out_names=tuple(out_names),
            lowering_input_output_aliases=(),
            sim_require_finite=True,
            sim_require_nnan=True,
            nc=nc,
        )
        return tuple(outs)

    devices = jax.devices()[:N_CORES]
    mesh = Mesh(np.asarray(devices), ("core",))
    sharded = jax.jit(
        shard_map(
            _body,
            mesh=mesh,
            in_specs=(PartitionSpec("core"),) * (n_params + n_outs),
            out_specs=(PartitionSpec("core"),) * n_outs,
            check_rep=False,
        ),
        donate_argnums=donate,
        keep_unused=True,
    )

    def prep_args(in_maps):
        concat_in = [
            np.concatenate([np.asarray(in_maps[c][nm]) for c in range(N_CORES)], axis=0)
            for nm in in_names
        ]
        concat_zero = [np.concatenate([z] * N_CORES, axis=0) for z in zero_outs]
        return concat_in, concat_zero

    def run(in_maps):
        concat_in, concat_zero = prep_args(in_maps)
        out_arrs = sharded(*concat_in, *concat_zero)
        full = np.asarray(out_arrs[0])
        return np.split(full, N_CORES, axis=0)

    run.sharded = sharded
    run.prep_args = prep_args
    run.mesh = mesh
    _RUNNER[t_steps] = run
    return run


def kernel(**inputs):
    inp = {k: np.asarray(v) for k, v in inputs.items()}
    for k in ("W_ih0", "W_hh0", "b_ih0", "b_hh0", "W_ih1", "W_hh1", "b_ih1",
              "b_hh1", "W_out", "b_out", "h0", "c0", "outputs"):
        assert k in inp, f"missing input {k}"

    shared = _prep_shared(inp)
    in_maps = []
    for c in range(N_CORES):
        m = dict(shared)
        m.update(_prep_core(inp, c))
        in_maps.append(m)

    run = _get_runner(_T)
    outs = run(in_maps)  # list of [T_FULL, BL] fp32 per core

    out_all = np.concatenate(outs[:N_GROUPS], axis=1)  # [T, B]
    targets = np.asarray(inp["outputs"]).T.astype(np.float32)  # [T, B]
    d = out_all[:_T].astype(np.float64) - targets[:_T].astype(np.float64)
    loss = np.sum(np.mean(d * d, axis=1))
    return np.float32(loss)
